# revision 6
# baseline (speedup 1.0000x reference)
"""Trainium2 Bass kernel for a 2-layer GraphNetwork (gnn_message_passing).

Strategy:
  - 16 graphs are partitioned across 8 cores (2 graphs per core). Every
    edge's receiver node lives on the edge's own core, so ALL segment
    reductions (per-node edge means, per-graph pooling) are core-local.
    No collectives are needed; the [16,128] output rows are gathered on
    the host.
  - Per core, nodes are bin-packed into NT tiles of 128 "slots"; each
    tile's incoming edges are padded to K0 chunks of 128. Segment-sums
    are computed on the tensor engine as one-hot selector matmuls
    (the one-hot [128e x 128n] block is built on-device from a column
    index via DVE is_equal against an iota tile).
  - Weights are replicated; biases are folded into matmuls via ones-rows.
  - bf16 inputs/intermediates, fp32 PSUM accumulation, fp32 final stage.
"""

import numpy as np
import ml_dtypes

import concourse.bass as bass
import concourse.tile as tile_mod
from concourse import tile
from concourse.bass_utils import run_bass_kernel_spmd
from concourse.vector_clock import ScopedClock

mybir = bass.mybir

N_NODES, N_EDGES, N_GRAPHS = 20000, 320000, 16
F_NODE, F_EDGE, F_GLOB = 64, 32, 16
N_CORES = 8
GPC = N_GRAPHS // N_CORES  # graphs per core = 2

BF16 = mybir.dt.bfloat16
F32 = mybir.dt.float32
npbf16 = ml_dtypes.bfloat16

# ---------------------------------------------------------------------------
# Workaround: CoreV3 codegen rejects the TileContext final drain when it
# carries more than one semaphore wait. Split the waits across extra no-ops.
_MAX_WAITS = 1


_ENGINE_WAIT_LIMIT = 1
_SPLIT_ENGINES = None  # set lazily


def _split_excess_waits(nc):
    """CoreV3 codegen caps per-instruction sem waits. Move excess waits
    onto same-engine no-ops inserted immediately before the offender."""
    global _SPLIT_ENGINES
    if _SPLIT_ENGINES is None:
        ET = mybir.EngineType
        _SPLIT_ENGINES = {ET.PE, ET.Activation, ET.DVE, ET.SP, ET.Pool}
    ctr = [0]
    for bass_bb in nc.bb_map.values():
        bb = bass_bb.bb
        il = bb.instructions
        out = []
        changed = False
        for inst in il:
            si = inst.sync_info
            waits = list(si.on_wait) if (si and si.on_wait) else []
            if len(waits) > _ENGINE_WAIT_LIMIT and inst.engine in _SPLIT_ENGINES:
                head, keep = waits[:-_ENGINE_WAIT_LIMIT], waits[-_ENGINE_WAIT_LIMIT:]
                for i in range(0, len(head), _ENGINE_WAIT_LIMIT):
                    nop = mybir.InstNoOp(name=f"waitsplit-{ctr[0]}", ins=[], outs=[])
                    ctr[0] += 1
                    nop.engine = inst.engine
                    nop.sync_info = mybir.SyncInfo(
                        on_wait=head[i : i + _ENGINE_WAIT_LIMIT], on_update=[]
                    )
                    nc.register_instruction(nop, overwrite=True)
                    out.append(nop)
                inst.sync_info = mybir.SyncInfo(
                    on_wait=keep, on_update=list(si.on_update or [])
                )
                changed = True
            out.append(inst)
        if changed:
            bb.instructions = out


def _split_drain_and_barrier(self, tick_clock, wait_clock):
    nc = self.nc
    _split_excess_waits(nc)
    drain_inst = nc.sync.drain()
    wait_clock.add_sem_waits(
        drain_inst.ins, ScopedClock({None: tick_clock.global_clock})
    )
    mi = drain_inst.ins
    waits = list(mi.sync_info.on_wait) if (mi.sync_info and mi.sync_info.on_wait) else []
    if len(waits) > _MAX_WAITS:
        upd = list(mi.sync_info.on_update) if mi.sync_info.on_update else []
        mi.sync_info = mybir.SyncInfo(on_wait=waits[:_MAX_WAITS], on_update=upd)
        for i in range(_MAX_WAITS, len(waits), _MAX_WAITS):
            nop = nc.sync.nop(nofuse=True)
            nop.ins.sync_info = mybir.SyncInfo(
                on_wait=waits[i : i + _MAX_WAITS], on_update=[]
            )
    nc.all_engine_barrier()
    assert self.sems is not None
    popped = nc._tile_sem_poison_stack.pop()
    assert popped is self._sem_poison
    nc.clear_and_free_semaphores(list(self.sems.allocated().values()))
    nc.all_engine_barrier()


tile_mod.TileContext._drain_and_barrier = _split_drain_and_barrier


# ---------------------------------------------------------------------------
# Host-side graph partitioning / layout


def _pack_core(node_ids, degs, nt, cap_e):
    """Greedy first-fit-decreasing: place nodes into nt tiles with at most
    128 nodes and cap_e incoming edges per tile. Returns list of node-id
    arrays (one per tile) or None if it does not fit."""
    order = np.argsort(-degs, kind="stable")
    tiles_n = [[] for _ in range(nt)]
    tile_ncnt = np.zeros(nt, np.int64)
    tile_ecnt = np.zeros(nt, np.int64)
    for j in order:
        d = degs[j]
        placed = False
        for t in range(nt):
            if tile_ncnt[t] < 128 and tile_ecnt[t] + d <= cap_e:
                tiles_n[t].append(node_ids[j])
                tile_ncnt[t] += 1
                tile_ecnt[t] += d
                placed = True
                break
        if not placed:
            return None
    return [np.array(t, dtype=np.int64) for t in tiles_n]


def _prepare(inputs):
    nf = np.asarray(inputs["node_feats"], np.float32)
    ef = np.asarray(inputs["edge_feats"], np.float32)
    glob = np.asarray(inputs["globals_"], np.float32)
    recv = np.asarray(inputs["receivers"]).astype(np.int64)
    ngraph = np.asarray(inputs["node_graph"]).astype(np.int64)

    cnt = np.bincount(recv, minlength=N_NODES).astype(np.int64)
    egraph = ngraph[recv]
    ncnt_g = np.bincount(ngraph, minlength=N_GRAPHS)
    ecnt_g = np.bincount(egraph, minlength=N_GRAPHS)

    node_core = ngraph // GPC
    edge_core = egraph // GPC

    core_nodes = [np.where(node_core == c)[0] for c in range(N_CORES)]
    NT = int(max((len(cn) + 127) // 128 for cn in core_nodes))

    packs = None
    K0 = max(1, int(max(np.bincount(edge_core, minlength=N_CORES)) + NT * 128 - 1)
             // (NT * 128))
    for k0 in range(K0, K0 + 12):
        trial = []
        ok = True
        for c in range(N_CORES):
            p = _pack_core(core_nodes[c], cnt[core_nodes[c]], NT, k0 * 128)
            if p is None:
                ok = False
                break
            trial.append(p)
        if ok:
            packs, K0 = trial, k0
            break
    assert packs is not None, "bin packing failed"

    NPAD = NT * 128
    EPAD = NT * K0 * 128

    # slot assignment per core
    w_np = {}
    slot_of_node = np.full(N_NODES, -1, np.int64)
    tile_of_node = np.full(N_NODES, -1, np.int64)
    in_maps = []
    for c in range(N_CORES):
        for t in range(NT):
            ids = packs[c][t]
            slot_of_node[ids] = t * 128 + np.arange(len(ids))
            tile_of_node[ids] = t

        # ---- edges
        eidx = np.where(edge_core == c)[0]
        et = tile_of_node[recv[eidx]]
        order = np.argsort(et, kind="stable")
        eidx = eidx[order]
        et = et[order]
        counts = np.bincount(et, minlength=NT)
        starts = np.concatenate([[0], np.cumsum(counts)[:-1]])
        off_in = np.arange(len(eidx)) - np.repeat(starts, counts)
        dst = et * (K0 * 128) + off_in
        assert (counts <= K0 * 128).all()

        eftT = np.zeros((33, EPAD), np.float32)
        eftT[:32, dst] = ef[eidx].T
        eftT[32, dst] = 1.0

        eg_loc = egraph[eidx] - c * GPC
        ghot = np.zeros((3, EPAD), np.float32)
        ghot[0, dst] = (eg_loc == 0)
        ghot[1, dst] = (eg_loc == 1)
        ghot[2, dst] = 1.0

        selidx = np.full(EPAD, -1.0, np.float32)
        selidx[dst] = (slot_of_node[recv[eidx]] % 128).astype(np.float32)
        # [NT, 128, K0] : chunk k, lane i  <- position (t*K0 + k)*128 + i
        sel3 = selidx.reshape(NT, K0, 128).transpose(0, 2, 1).copy()

        # ---- nodes
        slot_node = np.full(NPAD, -1, np.int64)
        for t in range(NT):
            ids = packs[c][t]
            slot_node[t * 128 : t * 128 + len(ids)] = ids
        valid = slot_node >= 0
        sn = np.where(valid, slot_node, 0)

        nftT = np.zeros((65, NPAD), np.float32)
        nftT[:64, valid] = nf[sn[valid]].T
        nftT[64, valid] = 1.0

        ng_loc = ngraph[sn] - c * GPC
        nhot = np.zeros((3, NPAD), np.float32)
        nhot[0] = valid * (ng_loc == 0)
        nhot[1] = valid * (ng_loc == 1)
        nhot[2] = valid * 1.0

        invc = np.zeros((NPAD, 1), np.float32)
        invc[valid, 0] = 1.0 / np.maximum(cnt[sn[valid]], 1)

        poolw = np.zeros((NPAD, 4), np.float32)
        for g in range(GPC):
            gid = c * GPC + g
            m = valid & (ng_loc == g)
            poolw[m, g] = 1.0 / max(ncnt_g[gid], 1)
            poolw[m, 2 + g] = cnt[sn[m]] / max(ecnt_g[gid], 1)

        globT = glob[c * GPC : (c + 1) * GPC].T.copy()  # [16, 2]

        in_maps.append(
            {
                "eft": eftT.astype(npbf16),
                "ghot": ghot.astype(npbf16),
                "selidx": sel3,
                "nft": nftT.astype(npbf16),
                "nhot": nhot.astype(npbf16),
                "invc": invc,
                "poolw": poolw.astype(npbf16),
                "globT": globT,
            }
        )

    # ---- replicated weights
    def bf(x):
        return np.ascontiguousarray(x).astype(npbf16)

    We1T = np.zeros((33, 256), np.float32)
    We1T[:32] = np.asarray(inputs["We1"], np.float32).T
    We1T[32] = np.asarray(inputs["be1"], np.float32)
    w_np["We1T"] = bf(We1T)

    We2 = np.asarray(inputs["We2"], np.float32)  # [128, 256]
    We2T = We2.T  # [256, 128]
    w_np["We2T"] = bf(np.concatenate([We2T[:128], We2T[128:]], axis=1))  # [128, 256]

    Wn1T = np.zeros((65, 256), np.float32)
    Wn1T[:64] = np.asarray(inputs["Wn1"], np.float32).T
    Wn1T[64] = np.asarray(inputs["bn1"], np.float32)
    w_np["Wn1T"] = bf(Wn1T)

    Win1T = np.asarray(inputs["Win1"], np.float32).T  # [256 fi, 256 fo]
    w_np["Win1T"] = bf(
        np.concatenate(
            [Win1T[:128, :128], Win1T[:128, 128:], Win1T[128:, :128], Win1T[128:, 128:]],
            axis=1,
        )
    )  # [128, 512] : cols b*256 + s*128

    Wn2T = np.asarray(inputs["Wn2"], np.float32).T  # [256, 128]
    w_np["Wn2T"] = bf(np.concatenate([Wn2T[:128], Wn2T[128:]], axis=1))  # [128, 256]
    w_np["Win2T"] = bf(np.asarray(inputs["Win2"], np.float32).T)  # [128, 128]

    w_np["Wg2T"] = np.asarray(inputs["Wg2"], np.float32).T.copy()  # [16, 128] f32
    w_np["Wng2T"] = np.asarray(inputs["Wng2"], np.float32).T.copy()
    w_np["be2r"] = bf(np.asarray(inputs["be2"], np.float32)[None, :])
    w_np["bn2r"] = bf(np.asarray(inputs["bn2"], np.float32)[None, :])

    w_np["WgnT"] = np.asarray(inputs["Wgn"], np.float32).T.copy()  # [128,128] f32
    w_np["WgeT"] = np.asarray(inputs["Wge"], np.float32).T.copy()
    w_np["WggT"] = np.asarray(inputs["Wgg"], np.float32).T.copy()  # [16, 128]
    w_np["bgr"] = np.asarray(inputs["bg"], np.float32)[None, :].copy()
    w_np["ones2"] = np.ones((1, 2), np.float32)
    w_np["iota"] = np.broadcast_to(
        np.arange(128, dtype=np.float32), (128, 128)
    ).copy()
    w_np["ident"] = np.eye(128, dtype=npbf16)
    w_np["ident2"] = np.eye(2, dtype=np.float32)

    for m in in_maps:
        m.update(w_np)
    return in_maps, NT, K0


# ---------------------------------------------------------------------------
# Device program (identical on all cores)


def _build(NT, K0):
    nc = bass.Bass()
    NPAD = NT * 128
    EPAD = NT * K0 * 128
    CW = K0 * 128  # edge columns per node-tile

    d_eft = nc.dram_tensor("eft", [33, EPAD], BF16, kind="ExternalInput")
    d_ghot = nc.dram_tensor("ghot", [3, EPAD], BF16, kind="ExternalInput")
    d_sel = nc.dram_tensor("selidx", [NT, 128, K0], F32, kind="ExternalInput")
    d_nft = nc.dram_tensor("nft", [65, NPAD], BF16, kind="ExternalInput")
    d_nhot = nc.dram_tensor("nhot", [3, NPAD], BF16, kind="ExternalInput")
    d_invc = nc.dram_tensor("invc", [NPAD, 1], F32, kind="ExternalInput")
    d_poolw = nc.dram_tensor("poolw", [NPAD, 4], BF16, kind="ExternalInput")
    d_globT = nc.dram_tensor("globT", [16, 2], F32, kind="ExternalInput")

    d_We1T = nc.dram_tensor("We1T", [33, 256], BF16, kind="ExternalInput")
    d_We2T = nc.dram_tensor("We2T", [128, 256], BF16, kind="ExternalInput")
    d_Wn1T = nc.dram_tensor("Wn1T", [65, 256], BF16, kind="ExternalInput")
    d_Win1T = nc.dram_tensor("Win1T", [128, 512], BF16, kind="ExternalInput")
    d_Wn2T = nc.dram_tensor("Wn2T", [128, 256], BF16, kind="ExternalInput")
    d_Win2T = nc.dram_tensor("Win2T", [128, 128], BF16, kind="ExternalInput")
    d_Wg2T = nc.dram_tensor("Wg2T", [16, 128], F32, kind="ExternalInput")
    d_Wng2T = nc.dram_tensor("Wng2T", [16, 128], F32, kind="ExternalInput")
    d_be2r = nc.dram_tensor("be2r", [1, 128], BF16, kind="ExternalInput")
    d_bn2r = nc.dram_tensor("bn2r", [1, 128], BF16, kind="ExternalInput")
    d_WgnT = nc.dram_tensor("WgnT", [128, 128], F32, kind="ExternalInput")
    d_WgeT = nc.dram_tensor("WgeT", [128, 128], F32, kind="ExternalInput")
    d_WggT = nc.dram_tensor("WggT", [16, 128], F32, kind="ExternalInput")
    d_bgr = nc.dram_tensor("bgr", [1, 128], F32, kind="ExternalInput")
    d_ones2 = nc.dram_tensor("ones2", [1, 2], F32, kind="ExternalInput")
    d_iota = nc.dram_tensor("iota", [128, 128], F32, kind="ExternalInput")
    d_ident = nc.dram_tensor("ident", [128, 128], BF16, kind="ExternalInput")
    d_ident2 = nc.dram_tensor("ident2", [2, 2], F32, kind="ExternalInput")

    d_out = nc.dram_tensor("out", [128, 2], F32, kind="ExternalOutput")

    Relu = mybir.ActivationFunctionType.Relu
    Copy = mybir.ActivationFunctionType.Copy

    with tile.TileContext(nc) as tc:
        with tc.tile_pool(name="wp", bufs=1) as wp:
            def wtile(dram, shape, dt):
                t = wp.tile(shape, dt, tag=dram.name)
                nc.sync.dma_start(t[:], dram[:])
                return t

            We1T = wtile(d_We1T, [33, 256], BF16)
            We2T = wtile(d_We2T, [128, 256], BF16)
            Wn1T = wtile(d_Wn1T, [65, 256], BF16)
            Win1T = wtile(d_Win1T, [128, 512], BF16)
            Wn2T = wtile(d_Wn2T, [128, 256], BF16)
            Win2T = wtile(d_Win2T, [128, 128], BF16)
            Wg2T = wtile(d_Wg2T, [16, 128], F32)
            Wng2T = wtile(d_Wng2T, [16, 128], F32)
            WgnT = wtile(d_WgnT, [128, 128], F32)
            WgeT = wtile(d_WgeT, [128, 128], F32)
            WggT = wtile(d_WggT, [16, 128], F32)
            bgr = wtile(d_bgr, [1, 128], F32)
            ones2 = wtile(d_ones2, [1, 2], F32)
            iota = wtile(d_iota, [128, 128], F32)
            ident = wtile(d_ident, [128, 128], BF16)
            ident2 = wtile(d_ident2, [2, 2], F32)
            globT = wtile(d_globT, [16, 2], F32)

            aggall = wp.tile([128, 384 * NT], BF16, tag="aggall")
            g2aug = wp.tile([3, 128], BF16, tag="g2aug")
            gnaug = wp.tile([3, 128], BF16, tag="gnaug")

            # --- per-core global projections gb = globals @ Wg2.T etc.
            with tc.tile_pool(name="psg", bufs=1, space=bass.MemorySpace.PSUM) as psg:
                pg = psg.tile([2, 256], F32, tag="pg")
                nc.tensor.matmul(pg[:, 0:128], globT[:], Wg2T[:], start=True, stop=True)
                nc.tensor.matmul(pg[:, 128:256], globT[:], Wng2T[:], start=True, stop=True)
                nc.scalar.activation(g2aug[0:2, :], pg[:, 0:128], Copy)
                nc.scalar.activation(gnaug[0:2, :], pg[:, 128:256], Copy)
                nc.sync.dma_start(g2aug[2:3, :], d_be2r[:])
                nc.sync.dma_start(gnaug[2:3, :], d_bn2r[:])

            # ----------------- edge phase -----------------
            with tc.tile_pool(name="ep", bufs=2) as ep, \
                 tc.tile_pool(name="esb", bufs=3) as esb, \
                 tc.tile_pool(name="psA", bufs=2, space=bass.MemorySpace.PSUM) as psA, \
                 tc.tile_pool(name="psB", bufs=2, space=bass.MemorySpace.PSUM) as psB, \
                 tc.tile_pool(name="psC", bufs=2, space=bass.MemorySpace.PSUM) as psC, \
                 tc.tile_pool(name="psAgg", bufs=2, space=bass.MemorySpace.PSUM) as psAgg:
                for t in range(NT):
                    eftt = ep.tile([33, CW], BF16, tag="eftt")
                    nc.sync.dma_start(eftt[:], d_eft[:, t * CW : (t + 1) * CW])
                    ght = ep.tile([3, CW], BF16, tag="ght")
                    nc.sync.dma_start(ght[:], d_ghot[:, t * CW : (t + 1) * CW])
                    sidx = ep.tile([128, K0], F32, tag="sidx")
                    nc.sync.dma_start(sidx[:], d_sel[t])
                    invc_t = ep.tile([128, 1], F32, tag="invc")
                    nc.sync.dma_start(invc_t[:], d_invc[t * 128 : (t + 1) * 128, :])

                    pagg = psAgg.tile([128, 384], F32, tag="pagg")
                    for k in range(K0):
                        sl = slice(k * 128, (k + 1) * 128)
                        pe1 = psA.tile([128, 256], F32, tag="pe1")
                        nc.tensor.matmul(pe1[:], eftt[:, sl], We1T[:], start=True, stop=True)
                        ef = esb.tile([128, 384], BF16, tag="ef")
                        nc.scalar.activation(ef[:, 0:256], pe1[:], Relu)

                        pe1T = psB.tile([128, 256], F32, tag="pe1T")
                        nc.tensor.matmul(pe1T[:, 0:128], We1T[:, 0:128], eftt[:, sl],
                                         start=True, stop=True)
                        nc.tensor.matmul(pe1T[:, 128:256], We1T[:, 128:256], eftt[:, sl],
                                         start=True, stop=True)
                        e1T = esb.tile([128, 256], BF16, tag="e1T")
                        nc.vector.tensor_scalar_max(e1T[:], pe1T[:], 0.0)

                        pe2 = psC.tile([128, 128], F32, tag="pe2")
                        nc.tensor.matmul(pe2[:], e1T[:, 0:128], We2T[:, 0:128], start=True, stop=False)
                        nc.tensor.matmul(pe2[:], e1T[:, 128:256], We2T[:, 128:256], start=False, stop=False)
                        nc.tensor.matmul(pe2[:], ght[:, sl], g2aug[:], start=False, stop=True)
                        nc.scalar.activation(ef[:, 256:384], pe2[:], Relu)

                        oh = esb.tile([128, 128], BF16, tag="oh")
                        nc.vector.tensor_scalar(
                            oh[:], iota[:], sidx[:, k : k + 1], None,
                            op0=mybir.AluOpType.is_equal,
                        )
                        nc.tensor.matmul(pagg[:], oh[:], ef[:],
                                         start=(k == 0), stop=(k == K0 - 1))

                    nc.scalar.activation(
                        aggall[:, t * 384 : (t + 1) * 384], pagg[:], Copy,
                        scale=invc_t[:],
                    )

            # ----------------- node phase -----------------
            with tc.tile_pool(name="np_", bufs=2) as np_, \
                 tc.tile_pool(name="nsb", bufs=3) as nsb, \
                 tc.tile_pool(name="npsA", bufs=2, space=bass.MemorySpace.PSUM) as npsA, \
                 tc.tile_pool(name="npsB", bufs=2, space=bass.MemorySpace.PSUM) as npsB, \
                 tc.tile_pool(name="npsC", bufs=2, space=bass.MemorySpace.PSUM) as npsC, \
                 tc.tile_pool(name="npsP", bufs=1, space=bass.MemorySpace.PSUM) as npsP:
                ppN = npsP.tile([2, 128], F32, tag="ppN")
                ppE = npsP.tile([2, 128], F32, tag="ppE")
                for t in range(NT):
                    aggsl = aggall[:, t * 384 : (t + 1) * 384]
                    pT = npsA.tile([128, 384], BF16, tag="pT")
                    nc.tensor.transpose(pT[:, 0:128], aggsl[:, 0:128], ident[:])
                    nc.tensor.transpose(pT[:, 128:256], aggsl[:, 128:256], ident[:])
                    nc.tensor.transpose(pT[:, 256:384], aggsl[:, 256:384], ident[:])
                    aggT = nsb.tile([128, 384], BF16, tag="aggT")
                    nc.vector.tensor_copy(aggT[:], pT[:])

                    nftt = np_.tile([65, 128], BF16, tag="nftt")
                    nc.sync.dma_start(nftt[:], d_nft[:, t * 128 : (t + 1) * 128])
                    nht = np_.tile([3, 128], BF16, tag="nht")
                    nc.sync.dma_start(nht[:], d_nhot[:, t * 128 : (t + 1) * 128])
                    pw = np_.tile([128, 4], BF16, tag="pw")
                    nc.sync.dma_start(pw[:], d_poolw[t * 128 : (t + 1) * 128, :])

                    pn1 = npsB.tile([128, 256], F32, tag="pn1")
                    for s in (0, 1):
                        ssl = slice(s * 128, (s + 1) * 128)
                        nc.tensor.matmul(pn1[:, ssl], Wn1T[:, ssl], nftt[:], start=True, stop=False)
                        nc.tensor.matmul(pn1[:, ssl], Win1T[:, s * 128 : s * 128 + 128],
                                         aggT[:, 0:128], start=False, stop=False)
                        nc.tensor.matmul(pn1[:, ssl], Win1T[:, 256 + s * 128 : 256 + s * 128 + 128],
                                         aggT[:, 128:256], start=False, stop=True)
                    n1T = nsb.tile([128, 256], BF16, tag="n1T")
                    nc.scalar.activation(n1T[:], pn1[:], Relu)

                    pn2 = npsC.tile([128, 128], F32, tag="pn2")
                    nc.tensor.matmul(pn2[:], n1T[:, 0:128], Wn2T[:, 0:128], start=True, stop=False)
                    nc.tensor.matmul(pn2[:], n1T[:, 128:256], Wn2T[:, 128:256], start=False, stop=False)
                    nc.tensor.matmul(pn2[:], aggT[:, 256:384], Win2T[:], start=False, stop=False)
                    nc.tensor.matmul(pn2[:], nht[:], gnaug[:], start=False, stop=True)
                    n2 = nsb.tile([128, 128], BF16, tag="n2")
                    nc.scalar.activation(n2[:], pn2[:], Relu)

                    nc.tensor.matmul(ppN[:], pw[:, 0:2], n2[:],
                                     start=(t == 0), stop=(t == NT - 1))
                    nc.tensor.matmul(ppE[:], pw[:, 2:4], aggsl[:, 256:384],
                                     start=(t == 0), stop=(t == NT - 1))

                # ----------------- final projection -----------------
                navg = nsb.tile([2, 128], F32, tag="navg")
                nc.scalar.activation(navg[:], ppN[:], Copy)
                eavg = nsb.tile([2, 128], F32, tag="eavg")
                nc.scalar.activation(eavg[:], ppE[:], Copy)

                ptr2 = npsA.tile([128, 4], F32, tag="pT")
                nc.tensor.transpose(ptr2[:, 0:2], navg[:], ident2[:])
                nc.tensor.transpose(ptr2[:, 2:4], eavg[:], ident2[:])
                nt2 = nsb.tile([128, 4], F32, tag="nt2")
                nc.scalar.activation(nt2[:], ptr2[:], Copy)

                pout = npsC.tile([128, 2], F32, tag="pn2")
                nc.tensor.matmul(pout[:], WgnT[:], nt2[:, 0:2], start=True, stop=False)
                nc.tensor.matmul(pout[:], WgeT[:], nt2[:, 2:4], start=False, stop=False)
                nc.tensor.matmul(pout[:], WggT[:], globT[:], start=False, stop=False)
                nc.tensor.matmul(pout[:], bgr[:], ones2[:], start=False, stop=True)
                outsb = nsb.tile([128, 2], F32, tag="outsb")
                nc.scalar.activation(outsb[:], pout[:], Copy)
                nc.sync.dma_start(d_out[:], outsb[:])

    return nc


_CACHE = {}


def _get_nc(NT, K0):
    key = (NT, K0)
    if key not in _CACHE:
        _CACHE[key] = _build(NT, K0)
    return _CACHE[key]


def _run(inputs, trace=False):
    in_maps, NT, K0 = _prepare(inputs)
    nc = _get_nc(NT, K0)
    res = run_bass_kernel_spmd(nc, in_maps, list(range(N_CORES)), trace=trace)
    out = np.zeros((N_GRAPHS, 128), np.float32)
    for c in range(N_CORES):
        r = np.asarray(res.results[c]["out"], np.float32)
        out[GPC * c] = r[:, 0]
        out[GPC * c + 1] = r[:, 1]
    return out, res


def kernel(**inputs):
    out, _ = _run(inputs, trace=False)
    return out


def kernel_traced(**inputs):
    return _run(inputs, trace=True)
